# revision 2
# baseline (speedup 1.0000x reference)
"""DiagonalSSM Trainium2 kernel.

Math (per batch b):
    ah = tanh(a)                              # [D]
    u_t = ah * u_{t-1} + x_t                  # scan along T, per channel d
    y   = b * u  (b folded into weights)
    out = y @ w_out.T + b_out
        = u @ (diag(b) @ w_out.T) + b_out

Sharding: data-parallel over B across the 8 NeuronCores (batch b -> core b).

Per-core dataflow:
  - DMA x[t-chunk, :] in natural [t, d] layout (contiguous 512KB blocks)
  - PE-transpose 128x128 blocks into [d, t] layout (fp32)
  - hardware tensor_tensor_scan along the free (t) axis per d-chunk,
    fp32 internal state, bf16 output
  - bf16 matmul u.T-slices (lhsT=[d,t]) @ w' ([d,e], w' = b[d]*w_out[e,d],
    built once on-chip via PE transpose + scaled PSUM copy)
  - bias add (b_out broadcast tile) on DVE, DMA out in natural [t, e] layout
"""

import numpy as np

import concourse.bacc as bacc
import concourse.mybir as mybir
from concourse import masks
from concourse.tile import TileContext
from concourse.bass_utils import run_bass_kernel_spmd

B, T, D = 8, 4096, 1024
P = 128
NDC = D // P          # 8 d-chunks
Q = 1024              # t elements per scan chunk (quarter)
NQ = T // Q           # 4 quarters
NTC = Q // P          # 8 t-chunks of 128 per quarter
NG = 2                # transpose groups per quarter (4 t-chunks each)
GW = Q // NG          # 512 t per group
EH = 512              # e-half width for matmul N
F32 = mybir.dt.float32
BF16 = mybir.dt.bfloat16
AF = mybir.ActivationFunctionType
ALU = mybir.AluOpType

# data0 of the scan as a stride-0 broadcast AP instead of a materialized
# [128, Q] tile
USE_BCAST_AP = True


def build_kernel():
    nc = bacc.Bacc("TRN2", target_bir_lowering=False, debug=False, num_devices=B)
    x = nc.declare_dram_parameter("x", [T, D], F32, isOutput=False)
    a = nc.declare_dram_parameter("a", [D], F32, isOutput=False)
    b = nc.declare_dram_parameter("b", [D], F32, isOutput=False)
    w = nc.declare_dram_parameter("w_out", [D, D], F32, isOutput=False)
    bo = nc.declare_dram_parameter("b_out", [D], F32, isOutput=False)
    out = nc.declare_dram_parameter("out", [T, D], F32, isOutput=True)

    with TileContext(nc) as tc:
        with (
            tc.tile_pool(name="const", bufs=1) as cpool,
            tc.tile_pool(name="stage", bufs=6) as stage_pool,
            tc.tile_pool(name="xT", bufs=2) as xT_pool,
            tc.tile_pool(name="y", bufs=2) as y_pool,
            tc.tile_pool(name="wT", bufs=1) as wT_pool,
            tc.tile_pool(name="outs", bufs=3) as out_pool,
            tc.tile_pool(name="psA", bufs=3, space="PSUM") as psA,
            tc.tile_pool(name="psB", bufs=4, space="PSUM") as psB,
        ):
            # ---------- constants ----------
            ident = cpool.tile([P, P], F32, tag="ident")
            masks.make_identity(nc, ident[:])

            # a, b as [128, 8]: column c holds elements c*128..c*128+127
            a_tile = cpool.tile([P, NDC], F32, tag="a_t")
            nc.sync.dma_start(out=a_tile[:], in_=a[:].rearrange("(c p) -> p c", p=P))
            ah_tile = cpool.tile([P, NDC], F32, tag="ah_t")
            nc.scalar.activation(ah_tile[:], a_tile[:], AF.Tanh)
            b_tile = cpool.tile([P, NDC], F32, tag="b_t")
            nc.sync.dma_start(out=b_tile[:], in_=b[:].rearrange("(c p) -> p c", p=P))

            # b_out broadcast to all partitions: [128, D]
            bo_row = cpool.tile([1, D], F32, tag="bo_row")
            nc.sync.dma_start(out=bo_row[:], in_=bo[:].rearrange("(o d) -> o d", o=1))
            bias_bc = cpool.tile([P, D], F32, tag="bias_bc")
            nc.gpsimd.partition_broadcast(bias_bc[:], bo_row[:])

            if not USE_BCAST_AP:
                ahbc = []
                for dc in range(NDC):
                    t_ = cpool.tile([P, Q], F32, tag=f"ahbc{dc}")
                    nc.gpsimd.memset(t_[:], 0.0)
                    nc.scalar.activation(
                        t_[:], t_[:], AF.Identity, bias=ah_tile[:, dc : dc + 1]
                    )
                    ahbc.append(t_)

            # ---------- weight prep: wT[dc] = b[d] * w_out[:, d-chunk].T (bf16) ----------
            wT = [wT_pool.tile([P, D], BF16, name=f"wT{dc}", tag=f"wT{dc}") for dc in range(NDC)]
            for half in range(2):
                wstages = []
                for k in range(4):
                    ec = half * 4 + k
                    ws = stage_pool.tile([P, D], F32, tag="stage")
                    nc.sync.dma_start(out=ws[:], in_=w[ec * P : (ec + 1) * P, :])
                    wstages.append(ws)
                for dc in range(NDC):
                    ps = psA.tile([P, EH], F32, tag="psA")
                    for k in range(4):
                        nc.tensor.transpose(
                            ps[:, k * P : (k + 1) * P],
                            wstages[k][:, dc * P : (dc + 1) * P],
                            ident[:],
                        )
                    # scaled copy PSUM -> SBUF bf16: w' = b[d] * w_out.T
                    nc.scalar.activation(
                        wT[dc][:, half * EH : (half + 1) * EH],
                        ps[:],
                        AF.Copy,
                        scale=b_tile[:, dc : dc + 1],
                    )

            # ---------- main loop over quarters ----------
            y_prev = None
            for q in range(NQ):
                # transpose x into [d, t] layout
                xTq = [xT_pool.tile([P, Q], F32, name=f"xT{dc}_{q}", tag=f"xT{dc}") for dc in range(NDC)]
                for g in range(NG):
                    xstages = []
                    for k in range(4):
                        t0 = q * Q + g * GW + k * P
                        xs = stage_pool.tile([P, D], F32, tag="stage")
                        nc.sync.dma_start(out=xs[:], in_=x[t0 : t0 + P, :])
                        xstages.append(xs)
                    for dc in range(NDC):
                        ps = psA.tile([P, EH], F32, tag="psA")
                        for k in range(4):
                            nc.tensor.transpose(
                                ps[:, k * P : (k + 1) * P],
                                xstages[k][:, dc * P : (dc + 1) * P],
                                ident[:],
                            )
                        nc.scalar.copy(xTq[dc][:, g * GW : (g + 1) * GW], ps[:])

                # scan per d-chunk: u_t = ah*u_{t-1} + x_t, bf16 out
                yq = [y_pool.tile([P, Q], BF16, name=f"y{dc}_{q}", tag=f"y{dc}") for dc in range(NDC)]
                for dc in range(NDC):
                    if USE_BCAST_AP:
                        data0 = ah_tile[:, dc : dc + 1].broadcast_to([P, Q])
                    else:
                        data0 = ahbc[dc][:]
                    initial = 0.0 if q == 0 else y_prev[dc][:, Q - 1 : Q]
                    nc.vector.tensor_tensor_scan(
                        out=yq[dc][:],
                        data0=data0,
                        data1=xTq[dc][:],
                        initial=initial,
                        op0=ALU.mult,
                        op1=ALU.add,
                    )

                # matmul: out[t, e] = sum_d u[d, t] * w'[d, e]  (+ b_out)
                for t_c in range(NTC):
                    ostage = out_pool.tile([P, D], F32, tag="ostage")
                    for eh in range(2):
                        ps = psB.tile([P, EH], F32, tag="psB")
                        for dc in range(NDC):
                            nc.tensor.matmul(
                                ps[:],
                                lhsT=yq[dc][:, t_c * P : (t_c + 1) * P],
                                rhs=wT[dc][:, eh * EH : (eh + 1) * EH],
                                start=(dc == 0),
                                stop=(dc == NDC - 1),
                            )
                        nc.vector.tensor_add(
                            ostage[:, eh * EH : (eh + 1) * EH],
                            ps[:],
                            bias_bc[:, eh * EH : (eh + 1) * EH],
                        )
                    t0 = q * Q + t_c * P
                    nc.sync.dma_start(out=out[t0 : t0 + P, :], in_=ostage[:])

                y_prev = yq

    nc.finalize()
    return nc


_NC = None


def _get_nc():
    global _NC
    if _NC is None:
        _NC = build_kernel()
    return _NC


def kernel(x, a, b, w_out, b_out):
    x = np.ascontiguousarray(x, dtype=np.float32)
    a = np.ascontiguousarray(a, dtype=np.float32)
    b = np.ascontiguousarray(b, dtype=np.float32)
    w_out = np.ascontiguousarray(w_out, dtype=np.float32)
    b_out = np.ascontiguousarray(b_out, dtype=np.float32)
    nc = _get_nc()
    in_maps = [
        {"x": x[c], "a": a, "b": b, "w_out": w_out, "b_out": b_out} for c in range(B)
    ]
    res = run_bass_kernel_spmd(nc, in_maps, list(range(B)))
    return np.stack([res.results[c]["out"] for c in range(B)], axis=0)


# revision 3
# speedup vs baseline: 87.4594x; 87.4594x over previous
"""DiagonalSSM Trainium2 kernel, v2: software-pipelined emission order.

Same math as v1 (see kernel.py docstring). Changes:
  - x quarter 0 is loaded/transposed before weight prep so PE starts sooner
  - transposes of quarter q+1 are emitted before matmuls of quarter q
    (per-engine streams are in-order; this removes PE stalls at quarter
    boundaries waiting on DVE scans)
  - XPOSE_MODE selects the PE transpose dtype path:
      "f32"  - plain fp32 transpose (2 cyc/row, exact)
      "f32r" - float32r-tagged transpose (1.5 cyc/row, bit-exact if HW agrees)
      "bf16" - pre-cast x to bf16 on ACT, 1 cyc/row, small accuracy loss
"""

import contextlib

import numpy as np

import concourse.bacc as bacc
import concourse.mybir as mybir
from concourse import masks
from concourse.tile import TileContext
from concourse.bass_utils import run_bass_kernel_spmd

B, T, D = 8, 4096, 1024
P = 128
NDC = D // P
Q = 1024
NQ = T // Q
NTC = Q // P
GW = 512              # t per transpose group (4 t-chunks -> one psum bank)
NG = Q // GW
EH = 512
F32 = mybir.dt.float32
F32R = mybir.dt.float32r
BF16 = mybir.dt.bfloat16
AF = mybir.ActivationFunctionType
ALU = mybir.AluOpType

XPOSE_MODE = "f32"


def build_kernel(loop_n=None):
    nc = bacc.Bacc("TRN2", target_bir_lowering=False, debug=False, num_devices=B)
    x = nc.declare_dram_parameter("x", [T, D], F32, isOutput=False)
    a = nc.declare_dram_parameter("a", [D], F32, isOutput=False)
    b = nc.declare_dram_parameter("b", [D], F32, isOutput=False)
    w = nc.declare_dram_parameter("w_out", [D, D], F32, isOutput=False)
    bo = nc.declare_dram_parameter("b_out", [D], F32, isOutput=False)
    out = nc.declare_dram_parameter("out", [T, D], F32, isOutput=True)

    with TileContext(nc) as tc:
        with (
            tc.tile_pool(name="const", bufs=1) as cpool,
            tc.tile_pool(name="stage", bufs=10) as stage_pool,
            tc.tile_pool(name="xT", bufs=2) as xT_pool,
            tc.tile_pool(name="y", bufs=2) as y_pool,
            tc.tile_pool(name="wT", bufs=1) as wT_pool,
            tc.tile_pool(name="outs", bufs=3) as out_pool,
            tc.tile_pool(name="psA", bufs=4, space="PSUM") as psA,
            tc.tile_pool(name="psB", bufs=4, space="PSUM") as psB,
        ):
          loop_cm = tc.For_i(0, loop_n, 1) if loop_n else contextlib.nullcontext()
          with loop_cm:
            # ---------- constants ----------
            ident = cpool.tile([P, P], F32, tag="ident")
            masks.make_identity(nc, ident[:])

            a_tile = cpool.tile([P, NDC], F32, tag="a_t")
            nc.sync.dma_start(out=a_tile[:], in_=a[:].rearrange("(c p) -> p c", p=P))
            ah_tile = cpool.tile([P, NDC], F32, tag="ah_t")
            nc.scalar.activation(ah_tile[:], a_tile[:], AF.Tanh)
            b_tile = cpool.tile([P, NDC], F32, tag="b_t")
            nc.sync.dma_start(out=b_tile[:], in_=b[:].rearrange("(c p) -> p c", p=P))

            bo_row = cpool.tile([1, D], F32, tag="bo_row")
            nc.sync.dma_start(out=bo_row[:], in_=bo[:].rearrange("(o d) -> o d", o=1))
            bias_bc = cpool.tile([P, D], F32, tag="bias_bc")
            nc.gpsimd.partition_broadcast(bias_bc[:], bo_row[:])

            def pe_transpose(ps_slice, in_slice):
                if XPOSE_MODE == "f32r":
                    nc.tensor.transpose(
                        ps_slice.bitcast(F32R),
                        in_slice.bitcast(F32R),
                        ident[:].bitcast(F32R),
                    )
                else:
                    nc.tensor.transpose(ps_slice, in_slice, ident[:])

            def load_and_transpose_quarter(q):
                """DMA x rows and PE-transpose into [d, t] tiles for quarter q."""
                xd = BF16 if XPOSE_MODE == "bf16" else F32
                xTq = [
                    xT_pool.tile([P, Q], xd, name=f"xT{dc}_{q}", tag=f"xT{dc}")
                    for dc in range(NDC)
                ]
                for g in range(NG):
                    xstages = []
                    for k in range(4):
                        t0 = q * Q + g * GW + k * P
                        xs = stage_pool.tile([P, D], F32, tag="stage")
                        nc.sync.dma_start(out=xs[:], in_=x[t0 : t0 + P, :])
                        if XPOSE_MODE == "bf16":
                            xb = stage_pool.tile([P, D], BF16, tag="stage_bf")
                            nc.scalar.copy(xb[:], xs[:])
                            xs = xb
                        xstages.append(xs)
                    for dc in range(NDC):
                        ps = psA.tile([P, GW], xd, tag="psA")
                        for k in range(4):
                            if XPOSE_MODE == "bf16":
                                nc.tensor.transpose(
                                    ps[:, k * P : (k + 1) * P],
                                    xstages[k][:, dc * P : (dc + 1) * P],
                                    ident_bf[:],
                                )
                            else:
                                pe_transpose(
                                    ps[:, k * P : (k + 1) * P],
                                    xstages[k][:, dc * P : (dc + 1) * P],
                                )
                        nc.scalar.copy(xTq[dc][:, g * GW : (g + 1) * GW], ps[:])
                return xTq

            if XPOSE_MODE == "bf16":
                ident_bf = cpool.tile([P, P], BF16, tag="ident_bf")
                nc.vector.tensor_copy(ident_bf[:], ident[:])

            # ---------- x quarter 0 first: get PE going ASAP ----------
            xT_cur = load_and_transpose_quarter(0)

            # ---------- weight prep ----------
            wT = [
                wT_pool.tile([P, D], BF16, name=f"wT{dc}", tag=f"wT{dc}")
                for dc in range(NDC)
            ]
            for half in range(2):
                wstages = []
                for k in range(4):
                    ec = half * 4 + k
                    ws = stage_pool.tile([P, D], F32, tag="stage")
                    nc.sync.dma_start(out=ws[:], in_=w[ec * P : (ec + 1) * P, :])
                    wstages.append(ws)
                for dc in range(NDC):
                    ps = psA.tile([P, EH], F32, tag="psA")
                    for k in range(4):
                        pe_transpose(
                            ps[:, k * P : (k + 1) * P],
                            wstages[k][:, dc * P : (dc + 1) * P],
                        )
                    nc.scalar.activation(
                        wT[dc][:, half * EH : (half + 1) * EH],
                        ps[:],
                        AF.Copy,
                        scale=b_tile[:, dc : dc + 1],
                    )

            # ---------- main pipelined loop ----------
            y_prev = None
            for q in range(NQ):
                # scans for quarter q
                yq = [
                    y_pool.tile([P, Q], BF16, name=f"y{dc}_{q}", tag=f"y{dc}")
                    for dc in range(NDC)
                ]
                for dc in range(NDC):
                    data0 = ah_tile[:, dc : dc + 1].broadcast_to([P, Q])
                    initial = 0.0 if q == 0 else y_prev[dc][:, Q - 1 : Q]
                    nc.vector.tensor_tensor_scan(
                        out=yq[dc][:],
                        data0=data0,
                        data1=xT_cur[dc][:],
                        initial=initial,
                        op0=ALU.mult,
                        op1=ALU.add,
                    )

                # next quarter's transposes BEFORE this quarter's matmuls
                if q + 1 < NQ:
                    xT_cur = load_and_transpose_quarter(q + 1)

                # matmuls + bias + store for quarter q
                for t_c in range(NTC):
                    ostage = out_pool.tile([P, D], F32, tag="ostage")
                    for eh in range(2):
                        ps = psB.tile([P, EH], F32, tag="psB")
                        for dc in range(NDC):
                            nc.tensor.matmul(
                                ps[:],
                                lhsT=yq[dc][:, t_c * P : (t_c + 1) * P],
                                rhs=wT[dc][:, eh * EH : (eh + 1) * EH],
                                start=(dc == 0),
                                stop=(dc == NDC - 1),
                            )
                        nc.vector.tensor_add(
                            ostage[:, eh * EH : (eh + 1) * EH],
                            ps[:],
                            bias_bc[:, eh * EH : (eh + 1) * EH],
                        )
                    t0 = q * Q + t_c * P
                    nc.sync.dma_start(out=out[t0 : t0 + P, :], in_=ostage[:])

                y_prev = yq

    nc.finalize()
    return nc


_NC = None


def _get_nc():
    global _NC
    if _NC is None:
        _NC = build_kernel()
    return _NC


def kernel(x, a, b, w_out, b_out):
    x = np.ascontiguousarray(x, dtype=np.float32)
    a = np.ascontiguousarray(a, dtype=np.float32)
    b = np.ascontiguousarray(b, dtype=np.float32)
    w_out = np.ascontiguousarray(w_out, dtype=np.float32)
    b_out = np.ascontiguousarray(b_out, dtype=np.float32)
    nc = _get_nc()
    in_maps = [
        {"x": x[c], "a": a, "b": b, "w_out": w_out, "b_out": b_out} for c in range(B)
    ]
    res = run_bass_kernel_spmd(nc, in_maps, list(range(B)))
    return np.stack([res.results[c]["out"] for c in range(B)], axis=0)
